# revision 1
# baseline (speedup 1.0000x reference)
"""Trainium2 Bass kernel for nn_DecLayer (GNN message-passing decoder layer).

Strategy
--------
Data-parallel over nodes: 10000 nodes are padded and split 1280 per core
across 8 NeuronCores.  Per core everything is computed in "transposed space"
(feature dim on SBUF partitions, edges/nodes on the free dim), which makes
every matmul a clean contraction with the weights as the stationary operand
and needs no on-chip transposes: the host hands the kernel h_E / h_V already
feature-major (same bytes, different layout).

Per node-group (GROUP nodes, supertiles of 512 edges):
  h1.T = W1e.T @ hE.T + W1v.T @ bcast(hV.T)       (PSUM accumulate, bf16)
  g1 = gelu(h1.T + b1)                            (ScalarE, bias per-partition)
  h2.T = W2.T @ g1; g2 = gelu(h2.T + b2)
  masked neighbor sum over K=32 on the free dim   (VectorE [mult +] reduce)
then per group: W3 matmul + rank-1 (b3 x masksum), residual, LayerNorm with
cross-partition stats via ones-matmuls (float32r), FFN (Win/Wout), LayerNorm2,
mask_V.  Output is produced feature-major and transposed back on the host.

If the runtime mask arrays are all ones (true for this problem's
setup_inputs), the kernel is compiled without the mask-broadcast matmul and
mask multiplies; the general path handles arbitrary masks.
"""

import json

import numpy as np
import ml_dtypes

import concourse.bass as bass
import concourse.mybir as mybir
import concourse.tile as tile

F32 = mybir.dt.float32
F32R = mybir.dt.float32r
BF16 = mybir.dt.bfloat16
AF = mybir.ActivationFunctionType
OP = mybir.AluOpType
AX = mybir.AxisListType

H = 128        # hidden
C = 384        # edge feature dim
K = 32         # neighbors
N_CORES = 8
GROUP = 128                      # nodes per group
ST_NODES = 16                    # nodes per supertile
ST_E = ST_NODES * K              # 512 edges per supertile
SCALE = 30.0
EPS = 1e-5


# ---------------------------------------------------------------------------
# walrus workaround: this build rejects >1 sync wait per instruction; split
# extra waits into standalone EventSemaphore instructions on the same engine
# (engines execute their stream in order, so semantics are preserved).
# ---------------------------------------------------------------------------
def _split_multi_waits(bir_json: bytes) -> bytes:
    m = json.loads(bir_json)
    for f in m.get("functions", []):
        for b in f.get("blocks", []):
            out = []
            for inst in b.get("instructions", []):
                si = inst.get("sync_info")
                waits = (si or {}).get("on_wait") or []
                if len(waits) > 1:
                    for j, w in enumerate(waits[:-1]):
                        out.append({
                            "debug": inst.get("debug", 0),
                            "engine": inst["engine"],
                            "ins": [], "outs": [],
                            "name": f"{inst['name']}_wsplit{j}",
                            "opcode": "EventSemaphore",
                            "sync_info": {"on_update": [], "on_wait": [w]},
                        })
                    si["on_wait"] = waits[-1:]
                out.append(inst)
            b["instructions"] = out
    return json.dumps(m).encode()


def _install_bir_fix():
    import concourse.bass_utils as bu
    import concourse.bass2jax as b2j
    if getattr(bu, "_wsplit_installed", False):
        return
    orig = bu.compile_bir_kernel

    def patched(bir_json, tmpdir, neff_name="file.neff"):
        return orig(_split_multi_waits(bir_json), tmpdir, neff_name)

    bu.compile_bir_kernel = patched
    b2j.compile_bir_kernel = patched
    bu._wsplit_installed = True


def _bf(x):
    return np.asarray(x, dtype=ml_dtypes.bfloat16)


def _r(ap):
    """fp32 matmul operand passthrough.  (float32r would double PE throughput
    on these small stats/broadcast matmuls, but this walrus build's verifier
    requires explicitly-rounded fp32r producers, so plain fp32 is used.)"""
    return ap


def build_nc(w, n_nodes, ones_masks=False):
    """Build the per-core Bass kernel. `w` holds the (host numpy) weights.

    n_nodes must be a multiple of GROUP.
    """
    assert n_nodes % GROUP == 0
    n_groups = n_nodes // GROUP
    n_edges = n_nodes * K

    nc = bass.Bass()

    hEt = nc.dram_tensor("hEt", [C, n_edges], F32, kind="ExternalInput")
    hVt = nc.dram_tensor("hVt", [H, n_nodes], F32, kind="ExternalInput")
    maskr = nc.dram_tensor("maskr", [1, n_edges], F32, kind="ExternalInput")
    msumb = nc.dram_tensor("msumb", [1, n_nodes], BF16, kind="ExternalInput")
    maskVr = nc.dram_tensor("maskVr", [1, n_nodes], F32, kind="ExternalInput")
    outt = nc.dram_tensor("outt", [H, n_nodes], F32, kind="ExternalOutput")

    # --- inline constants (weights are known at build time) ---
    W1 = w["W1_w"]
    w1e_h = np.concatenate([W1[H + 128 * j: H + 128 * (j + 1), :] for j in range(3)], axis=1)
    wout_h = np.concatenate([w["Wout_w"][128 * j: 128 * (j + 1), :] for j in range(4)], axis=1)
    w1e = nc.inline_tensor(_bf(w1e_h), name="w1e")
    w1v = nc.inline_tensor(_bf(W1[:H, :]), name="w1v")
    w2 = nc.inline_tensor(_bf(w["W2_w"]), name="w2")
    w3s = nc.inline_tensor(_bf(w["W3_w"] / SCALE), name="w3s")
    b3s_row = nc.inline_tensor(_bf(w["W3_b"] / SCALE).reshape(1, H), name="b3s")
    win = nc.inline_tensor(_bf(w["Win_w"]), name="win")
    wout = nc.inline_tensor(_bf(wout_h), name="wout")
    woutb_row = nc.inline_tensor(_bf(w["Wout_b"]).reshape(1, H), name="woutb")
    ones_row_b = nc.inline_tensor(np.ones((1, GROUP), ml_dtypes.bfloat16), name="onesrb")
    ones_row_f = nc.inline_tensor(np.ones((1, H), np.float32), name="onesrf")
    ones_col_f = nc.inline_tensor(np.ones((H, 1), np.float32), name="onescf")
    # bcast-matmul lhsT rows with folded constants:
    #   invH_row: mean bcast from raw column-sum (x 1/H)
    #   g1r/g2r: rstd bcast with LayerNorm gamma folded in (per-partition)
    invH_row = nc.inline_tensor(np.full((1, H), 1.0 / H, np.float32), name="invhr")
    g1_row = nc.inline_tensor(w["ln1_g"].astype(np.float32).reshape(1, H), name="g1row")
    g2_row = nc.inline_tensor(w["ln2_g"].astype(np.float32).reshape(1, H), name="g2row")
    # per-partition columns: b1, b2, ln1g, ln1b, ln2g, ln2b, winb0..3, eps
    cols_h = np.stack(
        [w["W1_b"], w["W2_b"], w["ln1_g"], w["ln1_b"], w["ln2_g"], w["ln2_b"]]
        + [w["Win_b"][128 * j: 128 * (j + 1)] for j in range(4)]
        + [np.full(H, EPS, np.float32)],
        axis=1,
    ).astype(np.float32)  # [128, 11]
    colsD = nc.inline_tensor(cols_h, name="cols")

    n_st = GROUP // ST_NODES

    # value specializations (checked against the actual weights at build time)
    b1z = not np.any(w["W1_b"])
    b2z = not np.any(w["W2_b"])
    b3z = not np.any(w["W3_b"])
    winbz = not np.any(w["Win_b"])
    woutbz = not np.any(w["Wout_b"])
    ln1bz = not np.any(w["ln1_b"])
    ln2bz = not np.any(w["ln2_b"])

    with tile.TileContext(nc) as tc:
        with (
            tc.tile_pool(name="const", bufs=1) as constp,
            tc.tile_pool(name="xe", bufs=2) as xep,
            tc.tile_pool(name="st", bufs=3) as stp,
            tc.tile_pool(name="grp", bufs=3) as grpp,
            tc.tile_pool(name="ps_st", bufs=2, space="PSUM") as pst,
            tc.tile_pool(name="ps_grp", bufs=2, space="PSUM") as pgr,
        ):
            # load constants once
            w1e_s = constp.tile([H, 3 * 128], BF16)
            nc.sync.dma_start(w1e_s[:], w1e[:])
            w1v_s = constp.tile([H, H], BF16)
            nc.sync.dma_start(w1v_s[:], w1v[:])
            w2_s = constp.tile([H, H], BF16)
            nc.sync.dma_start(w2_s[:], w2[:])
            w3s_s = constp.tile([H, H], BF16)
            nc.sync.dma_start(w3s_s[:], w3s[:])
            b3s_s = constp.tile([1, H], BF16)
            nc.sync.dma_start(b3s_s[:], b3s_row[:])
            win_s = constp.tile([H, 512], BF16)
            nc.sync.dma_start(win_s[:], win[:])
            wout_s = constp.tile([H, 512], BF16)
            nc.sync.dma_start(wout_s[:], wout[:])
            woutb_s = constp.tile([1, H], BF16)
            nc.sync.dma_start(woutb_s[:], woutb_row[:])
            onesrb_s = constp.tile([1, GROUP], BF16)
            nc.sync.dma_start(onesrb_s[:], ones_row_b[:])
            onesrf_s = constp.tile([1, H], F32)
            nc.sync.dma_start(onesrf_s[:], ones_row_f[:])
            onescf_s = constp.tile([H, 1], F32)
            nc.sync.dma_start(onescf_s[:], ones_col_f[:])
            invh_s = constp.tile([1, H], F32)
            nc.sync.dma_start(invh_s[:], invH_row[:])
            g1r_s = constp.tile([1, H], F32)
            nc.sync.dma_start(g1r_s[:], g1_row[:])
            g2r_s = constp.tile([1, H], F32)
            nc.sync.dma_start(g2r_s[:], g2_row[:])
            cols_s = constp.tile([H, 11], F32)
            nc.sync.dma_start(cols_s[:], colsD[:])

            def col(i):
                return cols_s[:, i:i + 1]

            def ln_stats(x):
                """Cross-partition mean/rstd of x [128, GROUP] via ones-matmuls."""
                sq = grpp.tile([H, GROUP], F32, tag="sq")
                nc.vector.tensor_tensor(sq[:], x[:], x[:], op=OP.mult)
                ps1 = pgr.tile([1, GROUP], F32, tag="gp")
                nc.tensor.matmul(ps1[:], _r(onescf_s[:]), _r(x[:]))
                ps2 = pgr.tile([1, GROUP], F32, tag="gp")
                nc.tensor.matmul(ps2[:], _r(onescf_s[:]), _r(sq[:]))
                mean = grpp.tile([1, GROUP], F32, tag="mean")
                nc.vector.tensor_scalar_mul(mean[:], ps1[:], 1.0 / H)
                m2 = grpp.tile([1, GROUP], F32, tag="m2")
                nc.vector.tensor_tensor(m2[:], mean[:], mean[:], op=OP.mult)
                var = grpp.tile([1, GROUP], F32, tag="var")
                # var = ps2/H - mean^2 in one fused op
                nc.vector.scalar_tensor_tensor(
                    var[:], ps2[:], 1.0 / H, m2[:],
                    op0=OP.mult, op1=OP.subtract,
                )
                sd = grpp.tile([1, GROUP], F32, tag="sd")
                nc.scalar.activation(sd[:], var[:], AF.Sqrt, bias=cols_s[0:1, 10:11])
                rstd = grpp.tile([1, GROUP], F32, tag="rstd")
                nc.vector.reciprocal(rstd[:], sd[:])
                return {"mean": mean, "rstd": rstd}

            def ln_finish(x, stats, g_i, b_i, extra_mul=None):
                # pmb = ones x mean ; prb = gamma x rstd (gamma folded in lhsT)
                pmb = pgr.tile([H, GROUP], F32, tag="gp")
                nc.tensor.matmul(pmb[:], _r(onesrf_s[:]), _r(stats["mean"][:]))
                prb = pgr.tile([H, GROUP], F32, tag="gp")
                nc.tensor.matmul(
                    prb[:], _r(g1r_s[:] if g_i == 2 else g2r_s[:]),
                    _r(stats["rstd"][:]),
                )
                t1 = grpp.tile([H, GROUP], F32, tag="t1")
                nc.vector.tensor_tensor(t1[:], x[:], pmb[:], op=OP.subtract)
                t2 = grpp.tile([H, GROUP], F32, tag="t2")
                nc.vector.tensor_tensor(t2[:], t1[:], prb[:], op=OP.mult)
                beta_zero = ln1bz if g_i == 2 else ln2bz
                if beta_zero:
                    xn = t2
                else:
                    xn = grpp.tile([H, GROUP], F32, tag="xn")
                    nc.vector.tensor_scalar(
                        xn[:], t2[:], scalar1=col(b_i), scalar2=None, op0=OP.add,
                    )
                if extra_mul is not None:
                    xm = grpp.tile([H, GROUP], F32, tag="xm")
                    nc.vector.tensor_tensor(xm[:], xn[:], extra_mul[:], op=OP.mult)
                    return xm
                return xn

            # ---------------- software-pipelined emission ----------------
            # Engines execute their instruction streams in order, so emission
            # order shapes the pipeline.  Per flat supertile index t we emit:
            #   A(t): W1 matmuls into ph1(t)          [PE]
            #   C(t-1): W2 matmul into ph2(t-1)       [PE]
            #   D(t-1): gelu2 (+mask mult)            [ACT/DVE]
            #   B(t): gelu1                           [ACT]
            #   E(t-1): neighbor-sum reduce           [DVE]
            # so PE never waits on the same supertile's activations.  Group
            # tails (W3/LN/FFN/LN2/store) are split into parts emitted one
            # per iteration while the next group's supertiles stream.
            gstate = {}   # g -> dict(xe, hv_f, hv_b, msum, mV, agg)
            ststate = {}  # t -> dict(ph1, ph2, g1, g2, pm)

            def group_loads(g):
                e0 = g * GROUP * K
                n0 = g * GROUP
                st = {}
                # small per-group loads first so they land before the bulk
                # h_E stream saturates the DMA engines
                st["hv_f"] = grpp.tile([H, GROUP], F32, tag="hv_f", name="hv_f")
                nc.sync.dma_start(st["hv_f"][:], hVt[:, n0:n0 + GROUP])
                st["hv_b"] = grpp.tile([H, GROUP], BF16, tag="hv_b", name="hv_b")
                nc.vector.tensor_copy(st["hv_b"][:], st["hv_f"][:])
                st["msum"] = grpp.tile([1, GROUP], BF16, tag="msum", name="msum")
                nc.sync.dma_start(st["msum"][:], msumb[:, n0:n0 + GROUP])
                st["xe"] = xep.tile([H, 3, GROUP * K], BF16, tag="xe", name="xe")
                # six half-chunk cast-DMAs per group: finer issue granularity
                # keeps the SWDGE stream busy and lets compute start earlier.
                # hh-major order: the first three transfers cover the first
                # half of every c-chunk, so supertile 0 starts after 3 of 6.
                half = GROUP * K // 2
                for hh in range(2):
                    for j in range(3):
                        nc.gpsimd.dma_start(
                            st["xe"][:, j, hh * half:(hh + 1) * half],
                            hEt[128 * j:128 * (j + 1),
                                e0 + hh * half:e0 + (hh + 1) * half],
                        )
                if not ones_masks:
                    st["mV"] = grpp.tile([1, GROUP], F32, tag="mV", name="mV")
                    nc.sync.dma_start(st["mV"][:], maskVr[:, n0:n0 + GROUP])
                st["agg"] = grpp.tile([H, GROUP], BF16, tag="agg", name="agg")
                gstate[g] = st

            mstate = {}

            def mrow_load(t):
                if ones_masks or t >= n_groups * n_st:
                    return
                mrow = stp.tile([1, ST_E], F32, tag="mrow")
                nc.sync.dma_start(mrow[:], maskr[:, t * ST_E:(t + 1) * ST_E])
                mstate[t] = mrow

            def stage_A(t):
                g, s = divmod(t, n_st)
                gs = gstate[g]
                c0 = s * ST_E
                st = {}
                ph1 = pst.tile([H, ST_E], F32, tag="ph1")
                for j in range(3):
                    nc.tensor.matmul(
                        ph1[:], w1e_s[:, 128 * j:128 * (j + 1)],
                        gs["xe"][:, j, c0:c0 + ST_E],
                        start=(j == 0), stop=False,
                    )
                hvs = gs["hv_b"][:, s * ST_NODES:(s + 1) * ST_NODES]
                nc.tensor.matmul(
                    ph1[:], w1v_s[:], hvs.broadcast_to([H, ST_NODES, K]),
                    start=False, stop=True,
                )
                st["ph1"] = ph1
                if not ones_masks:
                    pm = pst.tile([H, ST_E], F32, tag="pm")
                    nc.tensor.matmul(pm[:], _r(onesrf_s[:]), _r(mstate.pop(t)[:]))
                    st["pm"] = pm
                ststate[t] = st

            def stage_B(t):
                st = ststate[t]
                g1 = stp.tile([H, ST_E], BF16, tag="g1")
                nc.scalar.activation(g1[:], st["ph1"][:], AF.Gelu,
                                      bias=0.0 if b1z else col(0))
                st["g1"] = g1

            def stage_C(t):
                st = ststate[t]
                ph2 = pst.tile([H, ST_E], F32, tag="ph2")
                nc.tensor.matmul(ph2[:], w2_s[:], st["g1"][:])
                st["ph2"] = ph2

            def stage_D(t):
                st = ststate[t]
                g2 = stp.tile([H, ST_E], BF16, tag="g2")
                nc.scalar.activation(g2[:], st["ph2"][:], AF.Gelu,
                                      bias=0.0 if b2z else col(1))
                if ones_masks:
                    st["red"] = g2
                else:
                    g2m = stp.tile([H, ST_E], BF16, tag="g2m")
                    nc.vector.tensor_tensor(g2m[:], g2[:], st["pm"][:], op=OP.mult)
                    st["red"] = g2m

            def stage_E(t):
                g, s = divmod(t, n_st)
                st = ststate.pop(t)
                with nc.allow_low_precision("32-term neighbor sum fits bf16"):
                    nc.vector.reduce_sum(
                        gstate[g]["agg"][:, s * ST_NODES:(s + 1) * ST_NODES],
                        st["red"].rearrange("p (n k) -> p n k", k=K),
                        axis=AX.X,
                    )

            def tail_parts(g):
                gs = gstate[g]
                n0 = g * GROUP
                ctx = {}

                def p1():
                    pdh = pgr.tile([H, GROUP], F32, tag="gp")
                    nc.tensor.matmul(pdh[:], w3s_s[:], gs["agg"][:],
                                     start=True, stop=b3z)
                    if not b3z:
                        nc.tensor.matmul(pdh[:], b3s_s[:], gs["msum"][:],
                                         start=False, stop=True)
                    x = grpp.tile([H, GROUP], F32, tag="x")
                    nc.vector.tensor_tensor(x[:], gs["hv_f"][:], pdh[:], op=OP.add)
                    ctx["x"] = x

                def p2():
                    ctx["ln1"] = ln_stats(ctx["x"])

                def p3():
                    xln = ln_finish(ctx["x"], ctx["ln1"], 2, 3)
                    xlnb = grpp.tile([H, GROUP], BF16, tag="xlnb")
                    nc.vector.tensor_copy(xlnb[:], xln[:])
                    ctx["xln"], ctx["xlnb"] = xln, xlnb

                def p4():
                    gf = grpp.tile([H, 4, GROUP], BF16, tag="gf")
                    if winbz:
                        # zero bias: one [128, 4*GROUP] gelu over a single
                        # PSUM bank instead of four, amortizing ACT overhead
                        pf = pgr.tile([H, 4, GROUP], F32, tag="gp", name="pf")
                        for j in range(4):
                            nc.tensor.matmul(
                                pf[:, j, :], win_s[:, 128 * j:128 * (j + 1)],
                                ctx["xlnb"][:],
                            )
                        nc.scalar.activation(gf[:], pf[:], AF.Gelu, bias=0.0)
                    else:
                        for j in range(4):
                            pf = pgr.tile([H, GROUP], F32, tag="gp", name="pf")
                            nc.tensor.matmul(
                                pf[:], win_s[:, 128 * j:128 * (j + 1)],
                                ctx["xlnb"][:],
                            )
                            nc.scalar.activation(gf[:, j, :], pf[:], AF.Gelu,
                                                 bias=col(6 + j))
                    ctx["gf"] = gf

                def p5():
                    py = pgr.tile([H, GROUP], F32, tag="gp")
                    for j in range(4):
                        nc.tensor.matmul(
                            py[:], wout_s[:, 128 * j:128 * (j + 1)],
                            ctx["gf"][:, j, :], start=(j == 0),
                            stop=(woutbz and j == 3),
                        )
                    if not woutbz:
                        nc.tensor.matmul(py[:], woutb_s[:], onesrb_s[:],
                                         start=False, stop=True)
                    z = grpp.tile([H, GROUP], F32, tag="z")
                    nc.vector.tensor_tensor(z[:], ctx["xln"][:], py[:], op=OP.add)
                    ctx["z"] = z

                def p6():
                    ctx["ln2"] = ln_stats(ctx["z"])

                def p7():
                    if ones_masks:
                        outf = ln_finish(ctx["z"], ctx["ln2"], 4, 5)
                    else:
                        pmv = pgr.tile([H, GROUP], F32, tag="gp")
                        nc.tensor.matmul(pmv[:], _r(onesrf_s[:]), _r(gs["mV"][:]))
                        outf = ln_finish(ctx["z"], ctx["ln2"], 4, 5, extra_mul=pmv)
                    nc.sync.dma_start(outt[:, n0:n0 + GROUP], outf[:])
                    gstate.pop(g)

                return [p1, p2, p3, p4, p5, p6, p7]

            pending = []
            total = n_groups * n_st
            group_loads(0)
            mrow_load(0)
            mrow_load(1)
            for t in range(total + 1):
                g, s = divmod(t, n_st)
                if t < total:
                    if s == 1 and g + 1 < n_groups:
                        group_loads(g + 1)
                    mrow_load(t + 2)
                    stage_A(t)
                if t >= 1:
                    stage_C(t - 1)
                    stage_D(t - 1)
                if t < total:
                    stage_B(t)
                if t >= 1:
                    stage_E(t - 1)
                    if s == 0:
                        pending.extend(tail_parts(g - 1))
                if pending:
                    pending.pop(0)()
            while pending:
                pending.pop(0)()

    return nc


def _prep_core_inputs(h_V, h_E, mask_V, mask_attend, n_pad):
    """Host marshalling for one core's node slice (feature-major layouts)."""
    n = h_V.shape[0]
    hEt = np.zeros((C, n_pad * K), np.float32)
    hEt[:, : n * K] = h_E.reshape(n * K, C).T
    hVt = np.zeros((H, n_pad), np.float32)
    hVt[:, :n] = h_V.T
    maskr = np.zeros((1, n_pad * K), np.float32)
    maskr[:, : n * K] = mask_attend.reshape(1, n * K)
    msumb = np.zeros((1, n_pad), ml_dtypes.bfloat16)
    msumb[:, :n] = _bf(mask_attend.sum(axis=1, dtype=np.float32)).reshape(1, n)
    maskVr = np.zeros((1, n_pad), np.float32)
    maskVr[:, :n] = mask_V.reshape(1, n)
    return {
        "hEt": np.ascontiguousarray(hEt),
        "hVt": np.ascontiguousarray(hVt),
        "maskr": maskr,
        "msumb": msumb,
        "maskVr": maskVr,
    }


def kernel(h_V, h_E, mask_V, mask_attend,
           W1_w, W1_b, W2_w, W2_b, W3_w, W3_b,
           ln1_g, ln1_b, Win_w, Win_b, Wout_w, Wout_b, ln2_g, ln2_b):
    from concourse.bass_utils import run_bass_kernel_spmd

    _install_bir_fix()

    h_V = np.asarray(h_V, np.float32)
    h_E = np.asarray(h_E, np.float32)
    mask_V = np.asarray(mask_V, np.float32)
    mask_attend = np.asarray(mask_attend, np.float32)

    n_full = h_V.shape[0]
    per = (n_full + N_CORES - 1) // N_CORES          # 1250
    n_pad = ((per + GROUP - 1) // GROUP) * GROUP     # 1280

    w = dict(W1_w=W1_w, W1_b=W1_b, W2_w=W2_w, W2_b=W2_b, W3_w=W3_w, W3_b=W3_b,
             ln1_g=ln1_g, ln1_b=ln1_b, Win_w=Win_w, Win_b=Win_b,
             Wout_w=Wout_w, Wout_b=Wout_b, ln2_g=ln2_g, ln2_b=ln2_b)
    w = {k: np.asarray(v, np.float32) for k, v in w.items()}

    ones_masks = bool(np.all(mask_attend == 1.0) and np.all(mask_V == 1.0))
    nc = build_nc(w, n_pad, ones_masks=ones_masks)

    in_maps = []
    for c in range(N_CORES):
        lo, hi = c * per, min((c + 1) * per, n_full)
        in_maps.append(_prep_core_inputs(
            h_V[lo:hi], h_E[lo:hi], mask_V[lo:hi], mask_attend[lo:hi], n_pad
        ))

    res = run_bass_kernel_spmd(nc, in_maps, core_ids=list(range(N_CORES)))

    out = np.empty((n_full, H), np.float32)
    for c in range(N_CORES):
        lo, hi = c * per, min((c + 1) * per, n_full)
        out[lo:hi] = res.results[c]["outt"].T[: hi - lo]
    return out

